# revision 1
# baseline (speedup 1.0000x reference)
"""MultiHeadAttn (B=2, L=2048, D=512, 8 heads) on 8 TRN2 cores.

Sharding: data-parallel. Core i handles batch b=i//4, query rows
(i%4)*512..+512, all 8 heads. K/V projections over the full 2048 keys are
recomputed on each core (no collectives); gather = concat on host.

Per-core math (head-major permutation perm[n*64+j]=j*8+n applied on host):
  QT[hd,i] = (Wq[perm].T).T @ qT          (512x512)
  KT[hd,j] = ((Wk[perm]/temp).T).T @ kT   (512x2048)  temp folded into Wk
  V''[j,h*65+d] = hv[j, h*64+d], V''[j,h*65+64] = 1   (ones col -> softmax den)
  S_h = KT_h^T @ QT_h -> exp -> PV accumulates [O_h | den_h] in PSUM [65,512]
  O_h *= 1/den_h ;  x = sum_h O_h^T @ Wp_h + q ;  LayerNorm(ddof=1, eps=1e-9)

Matmul datapath in bf16 (attention contributes ~0.7% of output magnitude, so
bf16 noise is diluted ~100x); residual q + LayerNorm stay fp32.

Schedule: per-cell S->exp->PV software pipeline; V projection interleaved
into group 0's cell loop; PV accumulators double-buffered across groups
(PSUM banks: proj 2 + S 2 + acc 4 = 8).
"""

import numpy as np

B, L, D = 2, 2048, 512
NH, DH = 8, 64
ROWS = 512
TEMP = float(np.sqrt(512.0))
EPS = 1e-9

TRACE = False
TRACE_KW = {}
LAST_EXEC_NS = None
LAST_RESULTS = None

_prog = {}


def _ensure_path():
    try:
        import concourse.bass  # noqa: F401
    except ImportError:
        import sys
        sys.path.insert(0, "/opt/trn_rl_repo")


def _build(debug=False):
    _ensure_path()
    import concourse.bacc as bacc
    import concourse.mybir as mybir
    import concourse.tile as tile

    fp32 = mybir.dt.float32
    bf16 = mybir.dt.bfloat16
    AF = mybir.ActivationFunctionType
    ALU = mybir.AluOpType

    nc = bacc.Bacc("TRN2", target_bir_lowering=False, debug=False,
                   enable_asserts=True, num_devices=8)

    d_qT = nc.dram_tensor("qT", [D, ROWS], bf16, kind="ExternalInput").ap()
    d_qn = nc.dram_tensor("qnat", [ROWS, D], fp32, kind="ExternalInput").ap()
    d_kT = nc.dram_tensor("kT", [D, L], bf16, kind="ExternalInput").ap()
    d_vT = nc.dram_tensor("vT", [D, L], bf16, kind="ExternalInput").ap()
    d_lq = nc.dram_tensor("lhsTq", [D, D], bf16, kind="ExternalInput").ap()
    d_lk = nc.dram_tensor("lhsTk", [D, D], bf16, kind="ExternalInput").ap()
    d_rv = nc.dram_tensor("rhsv", [D, D], bf16, kind="ExternalInput").ap()
    d_wp = nc.dram_tensor("wp", [D, D], bf16, kind="ExternalInput").ap()
    d_sc = nc.dram_tensor("scale", [D], fp32, kind="ExternalInput").ap()
    d_of = nc.dram_tensor("offset", [D], fp32, kind="ExternalInput").ap()
    d_out = nc.dram_tensor("out", [ROWS, D], fp32, kind="ExternalOutput").ap()
    if debug:
        d_dqt = nc.dram_tensor("dbg_qt", [D, ROWS], bf16, kind="ExternalOutput").ap()
        d_dkt = nc.dram_tensor("dbg_kt", [D, L], bf16, kind="ExternalOutput").ap()
        d_dv2 = nc.dram_tensor("dbg_v2", [L, NH * 65], bf16, kind="ExternalOutput").ap()
        d_don = nc.dram_tensor("dbg_on", [D, ROWS], bf16, kind="ExternalOutput").ap()
        d_dx = nc.dram_tensor("dbg_x", [ROWS, D], fp32, kind="ExternalOutput").ap()
        d_dden = nc.dram_tensor("dbg_den", [NH, ROWS], fp32, kind="ExternalOutput").ap()
        d_dbc = nc.dram_tensor("dbg_bc", [64, ROWS], fp32, kind="ExternalOutput").ap()

    from contextlib import ExitStack
    with tile.TileContext(nc) as tc, ExitStack() as ctx:
        # ---- persistent SBUF tiles (one bufs=1 pool, distinct names) ----
        P = ctx.enter_context(tc.tile_pool(name="persist", bufs=1))
        DP = ctx.enter_context(tc.tile_pool(name="dscr", bufs=1, space="DRAM"))
        bscr = [DP.tile([1, ROWS], fp32, name=f"bscr{h}") for h in range(NH)]
        A = [P.tile([128, L], bf16, name=f"A{t}") for t in range(4)]       # kT
        AV = [P.tile([128, L], bf16, name=f"AV{t}") for t in range(4)]     # vT
        Wq1 = [P.tile([128, D], bf16, name=f"Wq{t}") for t in range(4)]
        Wk1 = [P.tile([128, D], bf16, name=f"Wk{t}") for t in range(4)]
        Wv1 = [P.tile([128, D], bf16, name=f"Wv{t}") for t in range(4)]
        U = [P.tile([128, D], bf16, name=f"U{t}") for t in range(4)]       # qT
        X = [P.tile([128, D], fp32, name=f"X{t}") for t in range(4)]       # ln scratch
        QT = [P.tile([128, ROWS], bf16, name=f"QT{t}") for t in range(4)]
        KT = [P.tile([128, L], bf16, name=f"KT{t}") for t in range(4)]
        V2 = [P.tile([128, NH * 65], bf16, name=f"V2_{j}") for j in range(16)]
        qn = [P.tile([128, D], fp32, name=f"qn{t}") for t in range(4)]
        WPp = [P.tile([128, D], bf16, name=f"WPp{g}") for g in range(4)]
        ONp = [P.tile([128, D], bf16, name=f"ONp{g}") for g in range(4)]
        ONs = [P.tile([64, ROWS], bf16, name=f"ONs{i}") for i in range(2)]
        rden = [P.tile([128, ROWS], fp32, name=f"rden{j}") for j in range(2)]
        bcd = [P.tile([64, ROWS], fp32, name=f"bcd{j}") for j in range(2)]
        scb = P.tile([128, D], fp32, name="scb")
        ofb = P.tile([128, D], fp32, name="ofb")
        stt = [P.tile([128, 6], fp32, name=f"stt{t}") for t in range(4)]
        mv = [P.tile([128, 2], fp32, name=f"mv{t}") for t in range(4)]
        sdt = [P.tile([128, 1], fp32, name=f"sdt{t}") for t in range(4)]
        rst = [P.tile([128, 1], fp32, name=f"rst{t}") for t in range(4)]

        # ---- input DMAs, priority-ordered + chunked for early compute ----
        for t in range(4):
            sl = slice(t * 128, (t + 1) * 128)
            nc.sync.dma_start(out=U[t], in_=d_qT[sl, :])
            nc.sync.dma_start(out=Wq1[t], in_=d_lq[sl, :])
        for t in range(4):
            sl = slice(t * 128, (t + 1) * 128)
            nc.sync.dma_start(out=Wk1[t], in_=d_lk[sl, :])
        for ls in range(4):
            cs = slice(ls * 512, (ls + 1) * 512)
            for dm in range(4):
                nc.sync.dma_start(out=A[dm][:, cs],
                                  in_=d_kT[dm * 128:(dm + 1) * 128, cs])
        for t in range(4):
            sl = slice(t * 128, (t + 1) * 128)
            nc.sync.dma_start(out=Wv1[t], in_=d_rv[sl, :])
        for jc in range(4):
            cs = slice(jc * 512, (jc + 1) * 512)
            for dm in range(4):
                nc.sync.dma_start(out=AV[dm][:, cs],
                                  in_=d_vT[dm * 128:(dm + 1) * 128, cs])
        for t in range(4):
            sl = slice(t * 128, (t + 1) * 128)
            nc.sync.dma_start(out=WPp[t], in_=d_wp[sl, :])
            nc.sync.dma_start(out=qn[t], in_=d_qn[sl, :])
        nc.sync.dma_start(out=scb, in_=d_sc.rearrange("(p f) -> p f", p=1).broadcast_to([128, D]))
        nc.sync.dma_start(out=ofb, in_=d_of.rearrange("(p f) -> p f", p=1).broadcast_to([128, D]))

        # ones columns of V'' (softmax denominator rides the PV matmul)
        for j in range(16):
            v3 = V2[j].rearrange("p (h c) -> p h c", h=NH)
            nc.vector.tensor_scalar(
                out=v3[:, :, 64:65],
                in0=Wq1[0][:, 0:8].rearrange("p (h c) -> p h c", c=1),
                scalar1=0.0, scalar2=1.0, op0=ALU.mult, op1=ALU.add)

        pp = ctx.enter_context(tc.tile_pool(name="pp", bufs=2, space="PSUM"))
        accp = ctx.enter_context(tc.tile_pool(name="accp", bufs=1, space="PSUM"))
        esp = ctx.enter_context(tc.tile_pool(name="esp", bufs=4))
        acc4 = [accp.tile([128, ROWS], fp32, name=f"acc{i}") for i in range(4)]

        # ---- Q projection -> QT [hd, 512] ----
        for t in range(4):
            pt = pp.tile([128, 1024], fp32, name=f"qp{t}", tag="ps")
            for dm in range(4):
                nc.tensor.matmul(pt[:, 0:ROWS], Wq1[dm][:, t * 128:(t + 1) * 128],
                                 U[dm], start=(dm == 0), stop=(dm == 3))
            nc.vector.tensor_copy(out=QT[t], in_=pt[:, 0:ROWS])

        # ---- K projection -> KT [hd, 2048] (temp pre-folded) ----
        for t in range(4):
            for lp in range(2):
                pt = pp.tile([128, 1024], fp32, name=f"kp{t}_{lp}", tag="ps")
                for half in range(2):
                    cs = slice((2 * lp + half) * 512, (2 * lp + half + 1) * 512)
                    for dm in range(4):
                        nc.tensor.matmul(pt[:, half * 512:(half + 1) * 512],
                                         Wk1[dm][:, t * 128:(t + 1) * 128],
                                         A[dm][:, cs], start=(dm == 0), stop=(dm == 3))
                nc.vector.tensor_copy(out=KT[t][:, lp * 1024:(lp + 1) * 1024], in_=pt)

        # ---- attention: per-cell S->exp->PV pipeline; Vproj fused into g=0 ----
        def s_pair(g, ks):
            wv = pp.tile([128, 1024], fp32, name=f"wv{g}_{ks}", tag="ps")
            for hh in range(2):
                p0 = hh * 64
                nc.tensor.matmul(wv[:, hh * ROWS:(hh + 1) * ROWS],
                                 KT[g][p0:p0 + 64, ks * 128:(ks + 1) * 128],
                                 QT[g][p0:p0 + 64, :], start=True, stop=True)
            es = esp.tile([128, 1024], bf16, name=f"es{g}_{ks}", tag="es")
            nc.scalar.activation(out=es, in_=wv, func=AF.Exp)
            return es

        def pv_cell(acc, h, ks, es, hh):
            nc.tensor.matmul(acc[0:65, :], V2[ks][:, h * 65:h * 65 + 65],
                             es[:, hh * ROWS:(hh + 1) * ROWS],
                             start=(ks == 0), stop=(ks == 15))

        def drain(g, acc_pair):
            h0 = 2 * g
            for j, h in enumerate((h0, h0 + 1)):
                acc = acc_pair[j]
                nc.vector.reciprocal(out=rden[j][64:65, :],
                                                 in_=acc[64:65, :])
                nc.sync.dma_start(out=bscr[h], in_=rden[j][64:65, :])
                nc.sync.dma_start(out=bcd[j], in_=bscr[h].broadcast_to([64, ROWS]))
                if debug:
                    nc.sync.dma_start(out=d_dden[h:h + 1, :], in_=rden[j][64:65, :])
                    if g == 0 and j == 0:
                        nc.sync.dma_start(out=d_dbc, in_=bcd[j])
                if j == 0:
                    nc.vector.tensor_tensor(out=ONp[g][0:64, :], in0=acc[0:64, :],
                                            in1=bcd[j], op=ALU.mult)
                else:
                    nc.vector.tensor_tensor(out=ONs[g % 2], in0=acc[0:64, :],
                                            in1=bcd[j], op=ALU.mult)
                    nc.sync.dma_start(out=ONp[g][64:128, :], in_=ONs[g % 2])

        for g in range(4):
            h0, h1 = 2 * g, 2 * g + 1
            acc_pair = (acc4[2 * (g % 2)], acc4[2 * (g % 2) + 1])
            prev = None
            for j in range(16):
                if g == 0:
                    # V projection for key rows j*128..+128 -> V'' (bf16)
                    v3 = V2[j].rearrange("p (h c) -> p h c", h=NH)
                    pt = pp.tile([128, 1024], fp32, name=f"vp{j}", tag="ps")
                    for dm in range(4):
                        nc.tensor.matmul(pt[:, 0:D], AV[dm][:, j * 128:(j + 1) * 128],
                                         Wv1[dm], start=(dm == 0), stop=(dm == 3))
                    nc.vector.tensor_copy(
                        out=v3[:, :, 0:64],
                        in_=pt[:, 0:D].rearrange("p (h c) -> p h c", h=NH))
                e01 = s_pair(g, j)
                if prev is not None:
                    pv_cell(acc_pair[0], h0, prev[0], prev[1], 0)
                    pv_cell(acc_pair[1], h1, prev[0], prev[1], 1)
                prev = (j, e01)
            pv_cell(acc_pair[0], h0, prev[0], prev[1], 0)
            pv_cell(acc_pair[1], h1, prev[0], prev[1], 1)
            drain(g, acc_pair)

        if debug:
            for t in range(4):
                nc.sync.dma_start(out=d_dqt[t * 128:(t + 1) * 128, :], in_=QT[t])
                nc.sync.dma_start(out=d_dkt[t * 128:(t + 1) * 128, :], in_=KT[t])
            for j in range(16):
                nc.sync.dma_start(out=d_dv2[j * 128:(j + 1) * 128, :], in_=V2[j])
            for g in range(4):
                nc.sync.dma_start(out=d_don[g * 128:(g + 1) * 128, :], in_=ONp[g])

        # ---- out projection + residual + LayerNorm ----
        for qs in range(4):
            xt = pp.tile([128, 1024], fp32, name=f"x{qs}", tag="ps")
            for g in range(4):
                nc.tensor.matmul(xt[:, 0:D], ONp[g][:, qs * 128:(qs + 1) * 128],
                                 WPp[g], start=(g == 0), stop=(g == 3))
            nc.vector.tensor_tensor(out=X[qs], in0=xt[:, 0:D], in1=qn[qs], op=ALU.add)
            if debug:
                nc.sync.dma_start(out=d_dx[qs * 128:(qs + 1) * 128, :], in_=X[qs])
            nc.vector.bn_stats(out=stt[qs], in_=X[qs])
            nc.vector.bn_aggr(out=mv[qs], in_=stt[qs])
            nc.scalar.activation(out=sdt[qs], in_=mv[qs][:, 1:2], func=AF.Sqrt,
                                 scale=float(D) / float(D - 1))
            nc.vector.tensor_scalar(out=rst[qs], in0=sdt[qs], scalar1=EPS,
                                    scalar2=None, op0=ALU.add)
            nc.vector.reciprocal(out=rst[qs], in_=rst[qs])
            nc.vector.scalar_tensor_tensor(
                out=X[qs], in0=X[qs], scalar=mv[qs][:, 0:1], in1=scb,
                op0=ALU.subtract, op1=ALU.mult)
            nc.vector.scalar_tensor_tensor(
                out=X[qs], in0=X[qs], scalar=rst[qs], in1=ofb,
                op0=ALU.mult, op1=ALU.add)
            nc.sync.dma_start(out=d_out[qs * 128:(qs + 1) * 128, :], in_=X[qs])

    nc.compile()
    return nc


def _get_prog():
    if "nc" not in _prog:
        _prog["nc"] = _build()
    return _prog["nc"]


def kernel(**inputs):
    global LAST_EXEC_NS, LAST_RESULTS
    _ensure_path()
    import ml_dtypes
    from concourse.bass_utils import run_bass_kernel_spmd
    bf = ml_dtypes.bfloat16

    q = np.asarray(inputs["q"], dtype=np.float32)
    k = np.asarray(inputs["k"], dtype=np.float32)
    v = np.asarray(inputs["v"], dtype=np.float32)
    Wq = np.asarray(inputs["Wq"], dtype=np.float32)
    Wk = np.asarray(inputs["Wk"], dtype=np.float32)
    Wv = np.asarray(inputs["Wv"], dtype=np.float32)
    Wp = np.asarray(inputs["Wp"], dtype=np.float32)
    scale = np.ascontiguousarray(inputs["scale"], dtype=np.float32)
    offset = np.ascontiguousarray(inputs["offset"], dtype=np.float32)

    # head-major permutation: perm[n*64+j] = j*8+n  (heads innermost in ref)
    perm = np.arange(D).reshape(DH, NH).T.ravel()
    lhsTq = np.ascontiguousarray(Wq[perm, :].T).astype(bf)
    lhsTk = np.ascontiguousarray((Wk[perm, :] / TEMP).T).astype(bf)
    rhsv = np.ascontiguousarray(Wv[perm, :].T).astype(bf)
    wp = np.ascontiguousarray(Wp[:, perm].T).astype(bf)

    in_maps = []
    for core in range(8):
        b, r0 = core // 4, (core % 4) * ROWS
        qblk = q[b, r0:r0 + ROWS, :]
        in_maps.append({
            "qT": np.ascontiguousarray(qblk.T).astype(bf),
            "qnat": np.ascontiguousarray(qblk),
            "kT": np.ascontiguousarray(k[b].T).astype(bf),
            "vT": np.ascontiguousarray(v[b].T).astype(bf),
            "lhsTq": lhsTq, "lhsTk": lhsTk, "rhsv": rhsv, "wp": wp,
            "scale": scale, "offset": offset,
        })

    nc = _get_prog()
    res = run_bass_kernel_spmd(nc, in_maps, core_ids=list(range(8)),
                               trace=TRACE, **TRACE_KW)
    LAST_EXEC_NS = res.exec_time_ns
    LAST_RESULTS = res

    out = np.empty((B, L, D), dtype=np.float32)
    for core in range(8):
        b, r0 = core // 4, (core % 4) * ROWS
        out[b, r0:r0 + ROWS, :] = res.results[core]["out"]
    return out



# revision 19
# speedup vs baseline: 1.2230x; 1.2230x over previous
"""MultiHeadAttn (B=2, L=2048, D=512, 8 heads) on 8 TRN2 cores.

Sharding: data-parallel. Core i handles batch b=i//4, query rows
(i%4)*512..+512, all 8 heads. K/V projections over the full 2048 keys are
recomputed on each core (no collectives); gather = concat on host.

v2: fp8(e4m3) DoubleRow matmuls for Q/K/V/out projections and PV
(0.5 cyc/row), S in bf16, exp on ACT with 1/temp folded into the
activation scale. The residual `+q` rides the out-projection PSUM via a
128*I identity matmul against bf16 q (LayerNorm is scale-invariant, so
the global 128x from the fp8 weight scaling cancels; the reference's
+1e-9 eps is a no-op at fp32 and is dropped). Softmax denominator rides
PV as a ones-column (col 64 of each head's V'' slab); 1/den is
partition-broadcast on gpsimd.

Scaling: weights stored x8 in fp8; QT/KT/V2 copies rescale by 1/8 to
natural units; ON stored x16 in fp8; out-proj result = 128*x, absorbed
by LN.

Per-head pipeline: S(h,jp) [2 bf16 matmuls, keytiles 2jp,2jp+1] -> exp
[fp8 es] -> PV-DoubleRow into acc[h%2]; V-proj interleaved into h=0's
cell loop; drain(h): recip -> partition_broadcast -> STT x16 -> ONpair.
Engine budget per core: ACT ~66us (exp, bottleneck), PE ~30us, DVE ~42us.
"""

import numpy as np

B, L, D = 2, 2048, 512
NH, DH = 8, 64
ROWS = 512
TEMP = float(np.sqrt(512.0))

TRACE = False
TRACE_KW = {}
LAST_EXEC_NS = None
LAST_RESULTS = None

_prog = {}


def _ensure_path():
    try:
        import concourse.bass  # noqa: F401
    except ImportError:
        import sys
        sys.path.insert(0, "/opt/trn_rl_repo")


def _build():
    _ensure_path()
    import concourse.bacc as bacc
    import concourse.mybir as mybir
    import concourse.tile as tile

    fp32 = mybir.dt.float32
    bf16 = mybir.dt.bfloat16
    fp8 = mybir.dt.float8e4
    AF = mybir.ActivationFunctionType
    ALU = mybir.AluOpType
    DRow = mybir.MatmulPerfMode.DoubleRow

    nc = bacc.Bacc("TRN2", target_bir_lowering=False, debug=False,
                   enable_asserts=True, num_devices=8)

    d_qT = nc.dram_tensor("qT8", [128, 2048], fp8, kind="ExternalInput").ap()
    d_kT = nc.dram_tensor("kT8", [128, 8192], fp8, kind="ExternalInput").ap()
    d_vT = nc.dram_tensor("vT8", [128, 8192], fp8, kind="ExternalInput").ap()
    d_wq = nc.dram_tensor("wq8", [128, 2048], fp8, kind="ExternalInput").ap()
    d_wk = nc.dram_tensor("wk8", [128, 2048], fp8, kind="ExternalInput").ap()
    d_wv = nc.dram_tensor("wv8", [128, 2048], fp8, kind="ExternalInput").ap()
    d_wp = nc.dram_tensor("wp8", [128, 2048], fp8, kind="ExternalInput").ap()
    d_qn = nc.dram_tensor("qnb", [ROWS, D], bf16, kind="ExternalInput").ap()
    d_id = nc.dram_tensor("ident", [128, 128], fp8, kind="ExternalInput").ap()
    d_sc = nc.dram_tensor("scale", [D], fp32, kind="ExternalInput").ap()
    d_of = nc.dram_tensor("offset", [D], fp32, kind="ExternalInput").ap()
    d_out = nc.dram_tensor("out", [ROWS, D], fp32, kind="ExternalOutput").ap()

    from contextlib import ExitStack
    with tile.TileContext(nc) as tc, ExitStack() as ctx:
        P = ctx.enter_context(tc.tile_pool(name="persist", bufs=1))
        qT8 = P.tile([128, 2048], fp8, name="qT8")
        kT8 = P.tile([128, 8192], fp8, name="kT8")
        vT8 = P.tile([128, 8192], fp8, name="vT8")
        wq8 = P.tile([128, 2048], fp8, name="wq8")
        wk8 = P.tile([128, 2048], fp8, name="wk8")
        wv8 = P.tile([128, 2048], fp8, name="wv8")
        wp8 = P.tile([128, 2048], fp8, name="wp8")
        qnb = [P.tile([128, D], bf16, name=f"qnb{t}") for t in range(4)]
        idt = P.tile([128, 128], fp8, name="idt")
        QT = [P.tile([128, ROWS], bf16, name=f"QT{t}") for t in range(4)]
        KT = [P.tile([128, L], bf16, name=f"KT{t}") for t in range(4)]
        V2 = [P.tile([128, NH * 2 * 80], fp8, name=f"V2_{j}") for j in range(8)]
        ONp = [P.tile([128, 2 * ROWS], fp8, name=f"ONp{s}") for s in range(2)]
        ONs = [P.tile([64, ROWS], fp8, name=f"ONs{i}") for i in range(2)]
        rdsb = [P.tile([1, ROWS], fp32, name=f"rdsb{i}") for i in range(2)]
        bcd = [P.tile([64, ROWS], fp32, name=f"bcd{i}") for i in range(2)]
        scb = P.tile([128, D], fp32, name="scb")
        ofb = P.tile([128, D], fp32, name="ofb")
        Xn = [P.tile([128, D], fp32, name=f"Xn{t}") for t in range(4)]
        stt = [P.tile([128, 6], fp32, name=f"stt{t}") for t in range(4)]
        mv = [P.tile([128, 2], fp32, name=f"mv{t}") for t in range(4)]
        sdt = [P.tile([128, 1], fp32, name=f"sdt{t}") for t in range(4)]
        rst = [P.tile([128, 1], fp32, name=f"rst{t}") for t in range(4)]

        # views of the DoubleRow-interleaved operands; every slice consumed
        # by a DR matmul is contiguous in free space (ISA requirement)
        qTv = qT8.rearrange("p (s i n) -> p s i n", s=2, i=2)
        kTv = kT8.rearrange("p (s c i k) -> p s c i k", s=2, c=4, i=2)
        vTv = vT8.rearrange("p (s j i k) -> p s j i k", s=2, j=16, i=2)
        wqv = wq8.rearrange("p (s t i m) -> p s t i m", s=2, t=4, i=2)
        wkv = wk8.rearrange("p (s t i m) -> p s t i m", s=2, t=4, i=2)
        wvv = wv8.rearrange("p (s i m) -> p s i m", s=2, i=2)
        wpv = wp8.rearrange("p (s i m) -> p s i m", s=2, i=2)
        d_kTv = d_kT.rearrange("p (s c i k) -> p s c i k", s=2, c=4, i=2)
        d_vTv = d_vT.rearrange("p (s j i k) -> p s j i k", s=2, j=16, i=2)

        # ---- input DMAs, priority-ordered + chunked for early compute ----
        nc.sync.dma_start(out=qT8, in_=d_qT)
        nc.sync.dma_start(out=wq8, in_=d_wq)
        nc.sync.dma_start(out=wk8, in_=d_wk)
        # kT first key-half for Kproj lp=0 of all t
        nc.sync.dma_start(out=kTv[:, :, 0:2], in_=d_kTv[:, :, 0:2])
        nc.sync.dma_start(out=wv8, in_=d_wv)
        nc.sync.dma_start(out=kTv[:, :, 2:4], in_=d_kTv[:, :, 2:4])
        for c in range(4):
            js = slice(c * 4, (c + 1) * 4)
            nc.sync.dma_start(out=vTv[:, :, js], in_=d_vTv[:, :, js])
        nc.sync.dma_start(out=idt, in_=d_id)
        nc.sync.dma_start(out=wp8, in_=d_wp)
        for t in range(4):
            nc.sync.dma_start(out=qnb[t], in_=d_qn[t * 128:(t + 1) * 128, :])
        nc.sync.dma_start(out=scb, in_=d_sc.rearrange("(p f) -> p f", p=1).broadcast_to([128, D]))
        nc.sync.dma_start(out=ofb, in_=d_of.rearrange("(p f) -> p f", p=1).broadcast_to([128, D]))

        # ones (c=64) + zero-pad (c=65:80) columns of V''; the ones column
        # makes the softmax denominator ride the PV matmul; per-half width 80
        # satisfies the DoubleRow ldweights stride%16==0 rule
        for j2 in range(8):
            v3 = V2[j2].rearrange("p (h i c) -> p h i c", h=NH, i=2)
            nc.vector.tensor_scalar(
                out=v3[:, :, :, 64:65],
                in0=wq8[:, 0:16].rearrange("p (h i c) -> p h i c", h=NH, i=2),
                scalar1=0.0, scalar2=1.0, op0=ALU.mult, op1=ALU.add)
            nc.vector.tensor_scalar(
                out=v3[:, :, :, 65:80],
                in0=wq8[:, 0:240].rearrange("p (h i c) -> p h i c", h=NH, i=2),
                scalar1=0.0, scalar2=None, op0=ALU.mult)

        pp = ctx.enter_context(tc.tile_pool(name="pp", bufs=2, space="PSUM"))
        accp = ctx.enter_context(tc.tile_pool(name="accp", bufs=1, space="PSUM"))
        esp = ctx.enter_context(tc.tile_pool(name="esp", bufs=4))
        acc2 = [accp.tile([128, ROWS], fp32, name=f"acc{i}") for i in range(2)]

        def qproj(t):
            pt = pp.tile([128, 1024], fp32, name=f"qp{t}", tag="ps")
            for s in range(2):
                nc.tensor.matmul(pt[:, 0:ROWS], wqv[:, s, t],
                                 qTv[:, s], start=(s == 0), stop=(s == 1),
                                 perf_mode=DRow)
            nc.vector.tensor_scalar(out=QT[t], in0=pt[:, 0:ROWS], scalar1=0.125,
                                    scalar2=None, op0=ALU.mult)

        def kproj(t, lp):
            # lp indexes 1024-key halves; kT chunks c are 512 keys
            cs = slice(lp * 1024, (lp + 1) * 1024)
            pt = pp.tile([128, 1024], fp32, name=f"kp{t}_{lp}", tag="ps")
            for cc in range(2):
                for s in range(2):
                    nc.tensor.matmul(pt[:, cc * 512:(cc + 1) * 512],
                                     wkv[:, s, t], kTv[:, s, 2 * lp + cc],
                                     start=(s == 0), stop=(s == 1),
                                     perf_mode=DRow)
            nc.vector.tensor_scalar(out=KT[t][:, cs], in0=pt, scalar1=0.125,
                                    scalar2=None, op0=ALU.mult)

        def vproj(j):
            # keytile j (128 keys) -> V''[j//2][:, h, j%2, 0:64], natural hv
            pt = pp.tile([128, 1024], fp32, name=f"vp{j}", tag="ps")
            for s in range(2):
                nc.tensor.matmul(pt[:, 0:D], vTv[:, s, j],
                                 wvv[:, s], start=(s == 0), stop=(s == 1),
                                 perf_mode=DRow)
            v3 = V2[j // 2].rearrange("p (h i c) -> p h i c", h=NH, i=2)
            nc.vector.tensor_scalar(
                out=v3[:, :, j % 2, 0:64],
                in0=pt[:, 0:D].rearrange("p (h c) -> p h c", h=NH),
                scalar1=0.125, scalar2=None, op0=ALU.mult)

        def s_exp(h, jp):
            # S for head h, keytiles 2jp, 2jp+1 -> exp -> es fp8 [128,(2,512)]
            g, p0 = h // 2, (h % 2) * 64
            wv_ps = pp.tile([128, 1024], fp32, name=f"wv{h}_{jp}", tag="ps")
            for u in range(2):
                kt = 2 * jp + u
                nc.tensor.matmul(wv_ps[:, u * ROWS:(u + 1) * ROWS],
                                 KT[g][p0:p0 + 64, kt * 128:(kt + 1) * 128],
                                 QT[g][p0:p0 + 64, :], start=True, stop=True)
            es = esp.tile([128, 1024], fp8, name=f"es{h}_{jp}", tag="es")
            nc.scalar.activation(out=es, in_=wv_ps, func=AF.Exp, scale=1.0 / TEMP)
            return es

        def pv(h, jp, es):
            acc = acc2[h % 2]
            nc.tensor.matmul(
                acc[0:80, :],
                V2[jp].rearrange("p (h i c) -> p h i c", h=NH, i=2)[:, h],
                es.rearrange("p (i n) -> p i n", i=2),
                start=(jp == 0), stop=(jp == 7), perf_mode=DRow)

        def drain(h):
            # ON = 16 * O / den -> ONp[s][po:po+64, :, i, :] fp8
            acc = acc2[h % 2]
            s, i, po = h // 4, (h // 2) % 2, (h % 2) * 64
            o4 = ONp[s].rearrange("p (q i n) -> p q i n", q=4, i=2)
            accv = acc[0:64, :].rearrange("p (q n) -> p q n", q=4)
            bcv = bcd[h % 2].rearrange("p (q n) -> p q n", q=4)
            nc.vector.reciprocal(out=rdsb[h % 2], in_=acc[64:65, :])
            nc.gpsimd.partition_broadcast(bcd[h % 2], rdsb[h % 2])
            if po == 0:
                nc.vector.scalar_tensor_tensor(
                    out=o4[0:64, :, i, :], in0=accv, scalar=16.0,
                    in1=bcv, op0=ALU.mult, op1=ALU.mult)
            else:
                nc.vector.scalar_tensor_tensor(
                    out=ONs[h % 2], in0=acc[0:64, :], scalar=16.0,
                    in1=bcd[h % 2], op0=ALU.mult, op1=ALU.mult)
                nc.sync.dma_start(
                    out=o4[64:128, :, i, :],
                    in_=ONs[h % 2].rearrange("p (q n) -> p q n", q=4))

        # ---- projections for t=0 (head 0/1), then the pipelined head loop ----
        qproj(0)
        kproj(0, 0)
        kproj(0, 1)
        vproj(0)
        vproj(1)

        for h in range(NH):
            prev = None
            for jp in range(8):
                if h == 0 and jp < 7:
                    vproj(2 * jp + 2)
                    vproj(2 * jp + 3)
                # stage remaining projections during early heads
                if jp == 4 and 0 <= h <= 2:
                    qproj(h + 1)
                    kproj(h + 1, 0)
                    kproj(h + 1, 1)
                es = s_exp(h, jp)
                if prev is not None:
                    pv(h, prev[0], prev[1])
                prev = (jp, es)
            pv(h, prev[0], prev[1])
            drain(h)

        # ---- out projection (+128*q via identity matmul) + LayerNorm ----
        for qs in range(4):
            xt = pp.tile([128, 1024], fp32, name=f"x{qs}", tag="ps")
            o4 = [ONp[s].rearrange("p (q i n) -> p q i n", q=4, i=2)
                  for s in range(2)]
            for s in range(2):
                nc.tensor.matmul(xt[:, 0:D],
                                 o4[s][:, qs],
                                 wpv[:, s], start=(s == 0), stop=False,
                                 perf_mode=DRow)
            nc.tensor.matmul(xt[:, 0:D], idt, qnb[qs], start=False, stop=True)
            nc.vector.bn_stats(out=stt[qs], in_=xt[:, 0:D])
            nc.vector.bn_aggr(out=mv[qs], in_=stt[qs])
            nc.scalar.activation(out=sdt[qs], in_=mv[qs][:, 1:2], func=AF.Sqrt,
                                 scale=float(D) / float(D - 1))
            nc.vector.reciprocal(out=rst[qs], in_=sdt[qs])
            nc.vector.scalar_tensor_tensor(
                out=Xn[qs], in0=xt[:, 0:D], scalar=mv[qs][:, 0:1], in1=scb,
                op0=ALU.subtract, op1=ALU.mult)
            nc.vector.scalar_tensor_tensor(
                out=Xn[qs], in0=Xn[qs], scalar=rst[qs], in1=ofb,
                op0=ALU.mult, op1=ALU.add)
            nc.sync.dma_start(out=d_out[qs * 128:(qs + 1) * 128, :], in_=Xn[qs])

    nc.compile()
    return nc


def _get_prog():
    if "nc" not in _prog:
        _prog["nc"] = _build()
    return _prog["nc"]


def _dr4(a):
    """[512, F] -> DoubleRow-interleaved [128, (s,i,F)] layout."""
    F = a.shape[1]
    return np.ascontiguousarray(
        a.reshape(2, 2, 128, F).transpose(2, 0, 1, 3).reshape(128, 4 * F))


def _dr4c(a, C):
    """[512, F] -> [128, (s, F//C chunks, i, C)]: DR pairs contiguous per
    C-column chunk."""
    F = a.shape[1]
    return np.ascontiguousarray(
        a.reshape(2, 2, 128, F // C, C).transpose(2, 0, 3, 1, 4).reshape(128, 4 * F))


def _prep(inputs):
    _ensure_path()
    import ml_dtypes
    bf = ml_dtypes.bfloat16
    f8 = ml_dtypes.float8_e4m3

    q = np.asarray(inputs["q"], dtype=np.float32)
    k = np.asarray(inputs["k"], dtype=np.float32)
    v = np.asarray(inputs["v"], dtype=np.float32)
    Wq = np.asarray(inputs["Wq"], dtype=np.float32)
    Wk = np.asarray(inputs["Wk"], dtype=np.float32)
    Wv = np.asarray(inputs["Wv"], dtype=np.float32)
    Wp = np.asarray(inputs["Wp"], dtype=np.float32)
    scale = np.ascontiguousarray(inputs["scale"], dtype=np.float32)
    offset = np.ascontiguousarray(inputs["offset"], dtype=np.float32)

    # head-major permutation: perm[n*64+j] = j*8+n  (heads innermost in ref)
    perm = np.arange(D).reshape(DH, NH).T.ravel()
    wq8 = _dr4c(8.0 * Wq[perm, :].T, 128).astype(f8)
    wk8 = _dr4c(8.0 * Wk[perm, :].T, 128).astype(f8)
    wv8 = _dr4(8.0 * Wv[perm, :].T).astype(f8)
    wp8 = _dr4(8.0 * Wp[:, perm].T).astype(f8)
    ident = (np.eye(128, dtype=np.float32) * 128.0).astype(f8)

    in_maps = []
    for core in range(8):
        b, r0 = core // 4, (core % 4) * ROWS
        qblk = q[b, r0:r0 + ROWS, :]
        in_maps.append({
            "qT8": _dr4(np.ascontiguousarray(qblk.T)).astype(f8),
            "kT8": _dr4c(np.ascontiguousarray(k[b].T), 512).astype(f8),
            "vT8": _dr4c(np.ascontiguousarray(v[b].T), 128).astype(f8),
            "wq8": wq8, "wk8": wk8, "wv8": wv8, "wp8": wp8,
            "qnb": np.ascontiguousarray(qblk).astype(bf),
            "ident": ident,
            "scale": scale, "offset": offset,
        })
    return in_maps


def emulate(inputs):
    """Numpy emulation of the per-core dataflow (layout/precision check)."""
    import ml_dtypes
    f32 = np.float32
    in_maps = _prep(inputs)

    def undr4(a8):
        F = a8.shape[1] // 4
        return a8.astype(f32).reshape(128, 2, 2, F).transpose(1, 2, 0, 3).reshape(512, F)

    def undr4c(a8, C):
        F = a8.shape[1] // 4
        return a8.astype(f32).reshape(128, 2, F // C, 2, C).transpose(
            1, 3, 0, 2, 4).reshape(512, F)

    out = np.empty((B, L, D), dtype=f32)
    for core in range(8):
        m = in_maps[core]
        b, r0 = core // 4, (core % 4) * ROWS
        wq = undr4c(m["wq8"], 128); wk = undr4c(m["wk8"], 128)
        wv = undr4(m["wv8"]); wp = undr4(m["wp8"])
        qT = undr4(m["qT8"]); kT = undr4c(m["kT8"], 512); vT = undr4c(m["vT8"], 128)
        QT = (wq.T @ qT) * 0.125                     # [hd, 512] natural hq
        KT = ((wk.T @ kT) * 0.125).astype(ml_dtypes.bfloat16).astype(f32)
        QT = QT.astype(ml_dtypes.bfloat16).astype(f32)
        Vn = ((vT.T @ wv) * 0.125).astype(ml_dtypes.float8_e4m3).astype(f32)
        ON = np.empty((D, ROWS), dtype=f32)
        for h in range(NH):
            S = KT[h * 64:(h + 1) * 64, :].T @ QT[h * 64:(h + 1) * 64, :]
            E = np.exp(S / TEMP).astype(ml_dtypes.float8_e4m3).astype(f32)
            O = Vn[:, h * 64:(h + 1) * 64].T @ E
            den = E.sum(axis=0)
            ON[h * 64:(h + 1) * 64, :] = 16.0 * O * (1.0 / den)[None, :]
        ON8 = ON.astype(ml_dtypes.float8_e4m3).astype(f32)
        qb = m["qnb"].astype(f32)
        x = ON8.T @ wp + 128.0 * qb                  # [i, e] = 128*x_true
        mu = x.mean(axis=-1, keepdims=True)
        sd = np.sqrt(x.var(axis=-1, keepdims=True) * D / (D - 1))
        out[b, r0:r0 + ROWS, :] = (inputs["scale"].astype(f32) * (x - mu) / sd
                                   + inputs["offset"].astype(f32))
    return out


def kernel(**inputs):
    global LAST_EXEC_NS, LAST_RESULTS
    _ensure_path()
    from concourse.bass_utils import run_bass_kernel_spmd

    in_maps = _prep(inputs)
    nc = _get_prog()
    res = run_bass_kernel_spmd(nc, in_maps, core_ids=list(range(8)),
                               trace=TRACE, **TRACE_KW)
    LAST_EXEC_NS = res.exec_time_ns
    LAST_RESULTS = res

    out = np.empty((B, L, D), dtype=np.float32)
    for core in range(8):
        b, r0 = core // 4, (core % 4) * ROWS
        out[b, r0:r0 + ROWS, :] = res.results[core]["out"]
    return out


# revision 24
# speedup vs baseline: 1.4478x; 1.1838x over previous
"""MultiHeadAttn (B=2, L=2048, D=512, 8 heads) on 8 TRN2 cores.

Sharding: data-parallel. Core i handles batch b=i//4, query rows
(i%4)*512..+512, all 8 heads. K/V projections over the full 2048 keys are
recomputed on each core (no collectives); gather = concat on host.

v3: fp8(e4m3) DoubleRow matmuls for Q/K/V/out projections and PV
(0.5 cyc/row), S in bf16, exp on ACT with 1/temp folded into the
activation scale. The residual `+q` rides the out-projection PSUM via a
128*I identity matmul against bf16 q (LayerNorm is scale-invariant, so
the global 128x from the fp8 weight scaling cancels; the reference's
+1e-9 eps is a no-op at fp32 and is dropped). Softmax denominator rides
PV as a ones-column (col 64 of each head's 80-wide V'' half-slab; width
80 satisfies the DR ldweights stride%16 rule); 1/den is
partition-broadcast on gpsimd.

Scaling: weights stored x8 in fp8; QT/KT/V2 copies rescale by 1/8 to
natural units; ON stored x16 in fp8; out-proj result = 128*x, absorbed
by LN.

Schedule: input DMAs split across the qSP/qAct HWDGE queues; a global
one-cell software pipeline S(cell+1) -> exp(cell+1) || PV(cell) across
all 64 (head, keypair) cells; separate PSUM pools for the S/exp
ping-pong (2x2 banks), projections (2x1), and PV accumulators (2x1).
Heads are processed in order [1,3,5,7,0,2,4,6] with Wp rows permuted to
match, so ONp[0] completes early (out-proj s=0 partials are taken to
bf16 mid-kernel) and the final drain writes its ON slab without a DMA
hop. LayerNorm: bn_stats/recip/STT1 on DVE, sqrt on ACT, STT2 on gpsimd.
"""

import numpy as np

B, L, D = 2, 2048, 512
NH, DH = 8, 64
ROWS = 512
TEMP = float(np.sqrt(512.0))
HO = [1, 3, 5, 7, 0, 2, 4, 6]  # head processing order

TRACE = False
TRACE_KW = {}
LAST_EXEC_NS = None
LAST_RESULTS = None

_prog = {}


def _ensure_path():
    try:
        import concourse.bass  # noqa: F401
    except ImportError:
        import sys
        sys.path.insert(0, "/opt/trn_rl_repo")


def _slot(u):
    """ONp slot for the u-th processed head: (s, i, partition offset)."""
    return u // 4, (u % 4) // 2, (1 - (u % 2)) * 64


def _build():
    _ensure_path()
    import concourse.bacc as bacc
    import concourse.mybir as mybir
    import concourse.tile as tile

    fp32 = mybir.dt.float32
    bf16 = mybir.dt.bfloat16
    fp8 = mybir.dt.float8e4
    AF = mybir.ActivationFunctionType
    ALU = mybir.AluOpType
    DRow = mybir.MatmulPerfMode.DoubleRow

    nc = bacc.Bacc("TRN2", target_bir_lowering=False, debug=False,
                   enable_asserts=True, num_devices=8)

    d_qT = nc.dram_tensor("qT8", [128, 2048], fp8, kind="ExternalInput").ap()
    d_kT = nc.dram_tensor("kT8", [128, 8192], fp8, kind="ExternalInput").ap()
    d_vT = nc.dram_tensor("vT8", [128, 8192], fp8, kind="ExternalInput").ap()
    d_wq = nc.dram_tensor("wq8", [128, 2048], fp8, kind="ExternalInput").ap()
    d_wk = nc.dram_tensor("wk8", [128, 2048], fp8, kind="ExternalInput").ap()
    d_wv = nc.dram_tensor("wv8", [128, 2048], fp8, kind="ExternalInput").ap()
    d_wp = nc.dram_tensor("wp8", [128, 2048], fp8, kind="ExternalInput").ap()
    d_qn = nc.dram_tensor("qnb", [ROWS, D], bf16, kind="ExternalInput").ap()
    d_id = nc.dram_tensor("ident", [128, 128], fp8, kind="ExternalInput").ap()
    d_sc = nc.dram_tensor("scale", [D], fp32, kind="ExternalInput").ap()
    d_of = nc.dram_tensor("offset", [D], fp32, kind="ExternalInput").ap()
    d_out = nc.dram_tensor("out", [ROWS, D], fp32, kind="ExternalOutput").ap()

    from contextlib import ExitStack
    with tile.TileContext(nc) as tc, ExitStack() as ctx:
        P = ctx.enter_context(tc.tile_pool(name="persist", bufs=1))
        qT8 = P.tile([128, 2048], fp8, name="qT8")
        kT8 = P.tile([128, 8192], fp8, name="kT8")
        vT8 = P.tile([128, 8192], fp8, name="vT8")
        wq8 = P.tile([128, 2048], fp8, name="wq8")
        wk8 = P.tile([128, 2048], fp8, name="wk8")
        wv8 = P.tile([128, 2048], fp8, name="wv8")
        wp8 = P.tile([128, 2048], fp8, name="wp8")
        qnb = [P.tile([128, D], bf16, name=f"qnb{t}") for t in range(4)]
        idt = P.tile([128, 128], fp8, name="idt")
        id1 = P.tile([128, 128], fp8, name="id1")
        QT = [P.tile([128, ROWS], bf16, name=f"QT{t}") for t in range(4)]
        KT = [P.tile([128, L], bf16, name=f"KT{t}") for t in range(4)]
        V2 = [P.tile([128, NH * 2 * 80], fp8, name=f"V2_{j}") for j in range(8)]
        ONp = [P.tile([128, 2 * ROWS], fp8, name=f"ONp{s}") for s in range(2)]
        ONs = [P.tile([64, ROWS], fp8, name=f"ONs{i}") for i in range(2)]
        rdsb = [P.tile([1, ROWS], fp32, name=f"rdsb{i}") for i in range(2)]
        bcd = [P.tile([64, ROWS], fp32, name=f"bcd{i}") for i in range(2)]
        xpart = [P.tile([128, D], bf16, name=f"xpart{t}") for t in range(4)]
        scb = P.tile([128, D], fp32, name="scb")
        ofb = P.tile([128, D], fp32, name="ofb")
        Xn = [P.tile([128, D], fp32, name=f"Xn{t}") for t in range(4)]
        stt = [P.tile([128, 6], fp32, name=f"stt{t}") for t in range(4)]
        mv = [P.tile([128, 2], fp32, name=f"mv{t}") for t in range(4)]
        sdt = [P.tile([128, 1], fp32, name=f"sdt{t}") for t in range(4)]
        rst = [P.tile([128, 1], fp32, name=f"rst{t}") for t in range(4)]

        # views of the DoubleRow-interleaved operands; every slice consumed
        # by a DR matmul is contiguous in free space and its per-half width
        # is a multiple of 16 bytes (ISA rules)
        qTv = qT8.rearrange("p (s i n) -> p s i n", s=2, i=2)
        kTv = kT8.rearrange("p (s c i k) -> p s c i k", s=2, c=4, i=2)
        vTv = vT8.rearrange("p (s j i k) -> p s j i k", s=2, j=16, i=2)
        wqv = wq8.rearrange("p (s t i m) -> p s t i m", s=2, t=4, i=2)
        wkv = wk8.rearrange("p (s t i m) -> p s t i m", s=2, t=4, i=2)
        wvv = wv8.rearrange("p (s i m) -> p s i m", s=2, i=2)
        wpv = wp8.rearrange("p (s i m) -> p s i m", s=2, i=2)
        d_kTv = d_kT.rearrange("p (s c i k) -> p s c i k", s=2, c=4, i=2)
        d_vTv = d_vT.rearrange("p (s j i k) -> p s j i k", s=2, j=16, i=2)

        # ---- input DMAs: split across the two HWDGE queues, priority first
        # qAct: the Q-projection chain + remaining weights
        nc.scalar.dma_start(out=wq8, in_=d_wq)
        nc.scalar.dma_start(out=qT8, in_=d_qT)
        nc.scalar.dma_start(out=wv8, in_=d_wv)
        nc.scalar.dma_start(out=idt, in_=d_id)
        nc.scalar.dma_start(out=wp8, in_=d_wp)
        for t in range(4):
            nc.scalar.dma_start(out=qnb[t], in_=d_qn[t * 128:(t + 1) * 128, :])
        nc.scalar.dma_start(out=scb, in_=d_sc.rearrange("(p f) -> p f", p=1).broadcast_to([128, D]))
        nc.scalar.dma_start(out=ofb, in_=d_of.rearrange("(p f) -> p f", p=1).broadcast_to([128, D]))
        # qSP: the K-projection chain + kT/vT bulk
        nc.sync.dma_start(out=wk8, in_=d_wk)
        nc.sync.dma_start(out=kTv[:, :, 0], in_=d_kTv[:, :, 0])
        nc.sync.dma_start(out=vTv[:, :, 0:4], in_=d_vTv[:, :, 0:4])
        nc.sync.dma_start(out=kTv[:, :, 1], in_=d_kTv[:, :, 1])
        nc.sync.dma_start(out=vTv[:, :, 4:8], in_=d_vTv[:, :, 4:8])
        nc.sync.dma_start(out=kTv[:, :, 2:4], in_=d_kTv[:, :, 2:4])
        nc.sync.dma_start(out=vTv[:, :, 8:16], in_=d_vTv[:, :, 8:16])

        # ones (c=64) + zero-pad (c=65:80) columns of V'' (on gpsimd:
        # SBUF-only, keeps DVE free for the PSUM evacuation copies)
        for j2 in range(8):
            v3 = V2[j2].rearrange("p (h i c) -> p h i c", h=NH, i=2)
            nc.gpsimd.tensor_scalar(
                out=v3[:, :, :, 64:65],
                in0=wq8[:, 0:16].rearrange("p (h i c) -> p h i c", h=NH, i=2),
                scalar1=0.0, scalar2=1.0, op0=ALU.mult, op1=ALU.add)
            nc.gpsimd.tensor_scalar(
                out=v3[:, :, :, 65:80],
                in0=wq8[:, 0:240].rearrange("p (h i c) -> p h i c", h=NH, i=2),
                scalar1=0.0, scalar2=None, op0=ALU.mult)
        # id1 = I (1.0), from idt = 128*I: exact in fp8
        nc.gpsimd.tensor_scalar(out=id1, in0=idt, scalar1=1.0 / 128.0,
                                scalar2=None, op0=ALU.mult)

        pp = ctx.enter_context(tc.tile_pool(name="pp", bufs=2, space="PSUM"))
        prj = ctx.enter_context(tc.tile_pool(name="prj", bufs=2, space="PSUM"))
        accp = ctx.enter_context(tc.tile_pool(name="accp", bufs=1, space="PSUM"))
        esp = ctx.enter_context(tc.tile_pool(name="esp", bufs=4))
        acc2 = [accp.tile([128, ROWS], fp32, name=f"acc{i}") for i in range(2)]

        def qproj(t):
            pt = prj.tile([128, ROWS], fp32, name=f"qp{t}", tag="pj")
            for s in range(2):
                nc.tensor.matmul(pt, wqv[:, s, t], qTv[:, s],
                                 start=(s == 0), stop=(s == 1), perf_mode=DRow)
            nc.vector.tensor_scalar(out=QT[t], in0=pt, scalar1=0.125,
                                    scalar2=None, op0=ALU.mult)

        def kproj(t, c):
            # 512-key chunk c
            pt = prj.tile([128, ROWS], fp32, name=f"kp{t}_{c}", tag="pj")
            for s in range(2):
                nc.tensor.matmul(pt, wkv[:, s, t], kTv[:, s, c],
                                 start=(s == 0), stop=(s == 1), perf_mode=DRow)
            nc.vector.tensor_scalar(out=KT[t][:, c * 512:(c + 1) * 512],
                                    in0=pt, scalar1=0.125,
                                    scalar2=None, op0=ALU.mult)

        def vproj(j):
            # keytile j (128 keys) -> V''[j//2][:, h, j%2, 0:64], natural hv;
            # copies alternate DVE / gpsimd to keep DVE off the critical path
            pt = prj.tile([128, ROWS], fp32, name=f"vp{j}", tag="pj")
            for s in range(2):
                nc.tensor.matmul(pt, vTv[:, s, j], wvv[:, s],
                                 start=(s == 0), stop=(s == 1), perf_mode=DRow)
            v3 = V2[j // 2].rearrange("p (h i c) -> p h i c", h=NH, i=2)
            nc.vector.tensor_scalar(
                out=v3[:, :, j % 2, 0:64],
                in0=pt.rearrange("p (h c) -> p h c", h=NH),
                scalar1=0.125, scalar2=None, op0=ALU.mult)

        def s_exp(h, jp):
            # S for head h, keytiles 2jp, 2jp+1 -> exp -> es fp8 [128,(2,512)]
            g, p0 = h // 2, (h % 2) * 64
            wv_ps = pp.tile([128, 1024], fp32, name=f"wv{h}_{jp}", tag="ps")
            for u in range(2):
                kt = 2 * jp + u
                nc.tensor.matmul(wv_ps[:, u * ROWS:(u + 1) * ROWS],
                                 KT[g][p0:p0 + 64, kt * 128:(kt + 1) * 128],
                                 QT[g][p0:p0 + 64, :], start=True, stop=True)
            es = esp.tile([128, 1024], fp8, name=f"es{h}_{jp}", tag="es")
            nc.scalar.activation(out=es, in_=wv_ps, func=AF.Exp, scale=1.0 / TEMP)
            return es

        def pv(u, h, jp, es):
            nc.tensor.matmul(
                acc2[u % 2][0:80, :],
                V2[jp].rearrange("p (h i c) -> p h i c", h=NH, i=2)[:, h],
                es.rearrange("p (i n) -> p i n", i=2),
                start=(jp == 0), stop=(jp == 7), perf_mode=DRow)

        def drain(u):
            # ON = 16 * O / den -> ONp[s][po:po+64, :, i, :] fp8
            acc = acc2[u % 2]
            s, i, po = _slot(u)
            o4 = ONp[s].rearrange("p (q i n) -> p q i n", q=4, i=2)
            accv = acc[0:64, :].rearrange("p (q n) -> p q n", q=4)
            bcv = bcd[u % 2].rearrange("p (q n) -> p q n", q=4)
            nc.vector.reciprocal(out=rdsb[u % 2], in_=acc[64:65, :])
            nc.gpsimd.partition_broadcast(bcd[u % 2], rdsb[u % 2])
            if po == 0:
                nc.vector.scalar_tensor_tensor(
                    out=o4[0:64, :, i, :], in0=accv, scalar=16.0,
                    in1=bcv, op0=ALU.mult, op1=ALU.mult)
            else:
                nc.vector.scalar_tensor_tensor(
                    out=ONs[(u // 2) % 2], in0=acc[0:64, :], scalar=16.0,
                    in1=bcd[u % 2], op0=ALU.mult, op1=ALU.mult)
                nc.sync.dma_start(
                    out=o4[64:128, :, i, :],
                    in_=ONs[(u // 2) % 2].rearrange("p (q n) -> p q n", q=4))

        def xpart0(qs):
            # out-proj s=0 partial (first 4 processed heads) -> bf16 SBUF
            o4 = ONp[0].rearrange("p (q i n) -> p q i n", q=4, i=2)
            pt = prj.tile([128, ROWS], fp32, name=f"x0_{qs}", tag="pj")
            nc.tensor.matmul(pt, o4[:, qs], wpv[:, 0],
                             start=True, stop=True, perf_mode=DRow)
            nc.vector.tensor_copy(out=xpart[qs], in_=pt)

        # ---- projections for the first processed head (1 -> t=0) ----
        qproj(0)
        for c in range(4):
            kproj(0, c)
        vproj(0)
        vproj(1)

        # ---- global one-cell software pipeline over all 64 cells ----
        cells = [(u, h, jp) for u, h in enumerate(HO) for jp in range(8)]
        prev = None
        for u, h, jp in cells:
            if u == 0 and jp >= 1:
                vproj(2 * jp)
                vproj(2 * jp + 1)
            # stage head-group t=u+1 projections across cells of head u
            if u <= 2:
                t = u + 1
                if jp == 2:
                    qproj(t)
                elif jp == 3:
                    kproj(t, 0)
                    kproj(t, 1)
                elif jp == 4:
                    kproj(t, 2)
                    kproj(t, 3)
            if u == 4 and jp in (1, 3, 5, 7):
                xpart0((jp - 1) // 2)
            es = s_exp(h, jp)
            if prev is not None:
                pv(*prev)
                if prev[2] == 7:
                    drain(prev[0])
            prev = (u, h, jp, es)
        pv(*prev)
        drain(prev[0])

        # ---- out projection tail: s=1 + 128*q + s=0 partial, then LN ----
        for qs in range(4):
            xt = prj.tile([128, ROWS], fp32, name=f"x{qs}", tag="pj")
            o4 = ONp[1].rearrange("p (q i n) -> p q i n", q=4, i=2)
            nc.tensor.matmul(xt, o4[:, qs], wpv[:, 1],
                             start=True, stop=False, perf_mode=DRow)
            nc.tensor.matmul(xt, idt, qnb[qs], start=False, stop=False)
            nc.tensor.matmul(xt, id1, xpart[qs], start=False, stop=True)
            nc.vector.bn_stats(out=stt[qs], in_=xt)
            nc.vector.bn_aggr(out=mv[qs], in_=stt[qs])
            nc.scalar.activation(out=sdt[qs], in_=mv[qs][:, 1:2], func=AF.Sqrt,
                                 scale=float(D) / float(D - 1))
            nc.vector.reciprocal(out=rst[qs], in_=sdt[qs])
            nc.vector.scalar_tensor_tensor(
                out=Xn[qs], in0=xt, scalar=mv[qs][:, 0:1], in1=scb,
                op0=ALU.subtract, op1=ALU.mult)
            nc.vector.scalar_tensor_tensor(
                out=Xn[qs], in0=Xn[qs], scalar=rst[qs], in1=ofb,
                op0=ALU.mult, op1=ALU.add)
            nc.sync.dma_start(out=d_out[qs * 128:(qs + 1) * 128, :], in_=Xn[qs])

    nc.compile()
    return nc


def _get_prog():
    if "nc" not in _prog:
        _prog["nc"] = _build()
    return _prog["nc"]


def _dr4(a):
    """[512, F] -> DoubleRow-interleaved [128, (s,i,F)] layout."""
    F = a.shape[1]
    return np.ascontiguousarray(
        a.reshape(2, 2, 128, F).transpose(2, 0, 1, 3).reshape(128, 4 * F))


def _dr4c(a, C):
    """[512, F] -> [128, (s, F//C chunks, i, C)]: DR pairs contiguous per
    C-column chunk."""
    F = a.shape[1]
    return np.ascontiguousarray(
        a.reshape(2, 2, 128, F // C, C).transpose(2, 0, 3, 1, 4).reshape(128, 4 * F))


def _prep(inputs):
    _ensure_path()
    import ml_dtypes
    bf = ml_dtypes.bfloat16
    f8 = ml_dtypes.float8_e4m3

    q = np.asarray(inputs["q"], dtype=np.float32)
    k = np.asarray(inputs["k"], dtype=np.float32)
    v = np.asarray(inputs["v"], dtype=np.float32)
    Wq = np.asarray(inputs["Wq"], dtype=np.float32)
    Wk = np.asarray(inputs["Wk"], dtype=np.float32)
    Wv = np.asarray(inputs["Wv"], dtype=np.float32)
    Wp = np.asarray(inputs["Wp"], dtype=np.float32)
    scale = np.ascontiguousarray(inputs["scale"], dtype=np.float32)
    offset = np.ascontiguousarray(inputs["offset"], dtype=np.float32)

    # head-major permutation: perm[n*64+j] = j*8+n  (heads innermost in ref)
    perm = np.arange(D).reshape(DH, NH).T.ravel()
    wq8 = _dr4c(8.0 * Wq[perm, :].T, 128).astype(f8)
    wk8 = _dr4c(8.0 * Wk[perm, :].T, 128).astype(f8)
    wv8 = _dr4(8.0 * Wv[perm, :].T).astype(f8)
    # Wp rows reordered to match the ONp slot layout of the HO head order
    wpp = 8.0 * Wp[:, perm].T
    wprows = np.empty_like(wpp)
    for u, h in enumerate(HO):
        s, i, po = _slot(u)
        dst = (2 * s + i) * 128 + po
        wprows[dst:dst + 64] = wpp[h * 64:(h + 1) * 64]
    wp8 = _dr4(wprows).astype(f8)
    ident = (np.eye(128, dtype=np.float32) * 128.0).astype(f8)

    in_maps = []
    for core in range(8):
        b, r0 = core // 4, (core % 4) * ROWS
        qblk = q[b, r0:r0 + ROWS, :]
        in_maps.append({
            "qT8": _dr4(np.ascontiguousarray(qblk.T)).astype(f8),
            "kT8": _dr4c(np.ascontiguousarray(k[b].T), 512).astype(f8),
            "vT8": _dr4c(np.ascontiguousarray(v[b].T), 128).astype(f8),
            "wq8": wq8, "wk8": wk8, "wv8": wv8, "wp8": wp8,
            "qnb": np.ascontiguousarray(qblk).astype(bf),
            "ident": ident,
            "scale": scale, "offset": offset,
        })
    return in_maps


def emulate(inputs):
    """Numpy emulation of the per-core dataflow (layout/precision check)."""
    import ml_dtypes
    f32 = np.float32
    in_maps = _prep(inputs)

    def undr4(a8):
        F = a8.shape[1] // 4
        return a8.astype(f32).reshape(128, 2, 2, F).transpose(1, 2, 0, 3).reshape(512, F)

    def undr4c(a8, C):
        F = a8.shape[1] // 4
        return a8.astype(f32).reshape(128, 2, F // C, 2, C).transpose(
            1, 3, 0, 2, 4).reshape(512, F)

    # inverse of the wp row reorder
    inv = np.empty(D, dtype=np.int64)
    for u, h in enumerate(HO):
        s, i, po = _slot(u)
        dst = (2 * s + i) * 128 + po
        inv[h * 64:(h + 1) * 64] = np.arange(dst, dst + 64)

    out = np.empty((B, L, D), dtype=f32)
    for core in range(8):
        m = in_maps[core]
        b, r0 = core // 4, (core % 4) * ROWS
        wq = undr4c(m["wq8"], 128); wk = undr4c(m["wk8"], 128)
        wv = undr4(m["wv8"]); wp = undr4(m["wp8"])[inv, :]
        qT = undr4(m["qT8"]); kT = undr4c(m["kT8"], 512); vT = undr4c(m["vT8"], 128)
        QT = ((wq.T @ qT) * 0.125).astype(ml_dtypes.bfloat16).astype(f32)
        KT = ((wk.T @ kT) * 0.125).astype(ml_dtypes.bfloat16).astype(f32)
        Vn = ((vT.T @ wv) * 0.125).astype(ml_dtypes.float8_e4m3).astype(f32)
        ON = np.empty((D, ROWS), dtype=f32)
        for h in range(NH):
            S = KT[h * 64:(h + 1) * 64, :].T @ QT[h * 64:(h + 1) * 64, :]
            E = np.exp(S / TEMP).astype(ml_dtypes.float8_e4m3).astype(f32)
            O = Vn[:, h * 64:(h + 1) * 64].T @ E
            den = E.sum(axis=0)
            ON[h * 64:(h + 1) * 64, :] = 16.0 * O * (1.0 / den)[None, :]
        ON8 = ON.astype(ml_dtypes.float8_e4m3).astype(f32)
        qb = m["qnb"].astype(f32)
        x = ON8.T @ wp + 128.0 * qb                  # [i, e] = 128*x_true
        mu = x.mean(axis=-1, keepdims=True)
        sd = np.sqrt(x.var(axis=-1, keepdims=True) * D / (D - 1))
        out[b, r0:r0 + ROWS, :] = (inputs["scale"].astype(f32) * (x - mu) / sd
                                   + inputs["offset"].astype(f32))
    return out


def kernel(**inputs):
    global LAST_EXEC_NS, LAST_RESULTS
    _ensure_path()
    from concourse.bass_utils import run_bass_kernel_spmd

    in_maps = _prep(inputs)
    nc = _get_prog()
    res = run_bass_kernel_spmd(nc, in_maps, core_ids=list(range(8)),
                               trace=TRACE, **TRACE_KW)
    LAST_EXEC_NS = res.exec_time_ns
    LAST_RESULTS = res

    out = np.empty((B, L, D), dtype=np.float32)
    for core in range(8):
        b, r0 = core // 4, (core % 4) * ROWS
        out[b, r0:r0 + ROWS, :] = res.results[core]["out"]
    return out
